# revision 16
# baseline (speedup 1.0000x reference)
"""
Trainium2 Bass kernel for nn_Attention_29265907155069.

Reference computation (B=4, N=2048, C=768, H=12, D=64):
    qkv = x @ qkv_w.T -> split to q,k,v per head
    attn = softmax(q @ k.T * D + mask * -1e6)
    out  = (attn @ v) re-concat -> @ proj_w.T + proj_b
Sharding: 8 cores = (batch b in 0..3) x (head-group hg in 0..1, 6 heads each).
Host sums the two head-group partials (row-sharded proj).

Per-core pipeline (v3):
  1. QKV: Q^T,K^T [d, n] and V [k, d] via PE matmuls (float32r); D=64 scale
     folded into Q weights on host. PSUM->SBUF copies on DVE (idle in ph 1).
  2. Scores S per (head a, q-tile, k-half) into PSUM [128, 1024] (two heads
     packed via K=64 row-tiling); identity matmul accumulates -2^20*mask.
  3. DVE reduce_max(negate) per half -> -m_half.  negm = -M = min of both,
     available right after half1's reduce at no extra latency.
  4. ACT exp: half0 biased by -m0 (fires immediately, frees its PSUM banks
     fast); half1 biased by negm.  Only half0 then needs the fs = exp(m0-M)
     renormalize multiply (DVE tensor_scalar, fp16 4x) - half the v1 cost.
  5. DMA xbar transpose per (head, q-tile): pns [128, 2048] -> PT.
  6. PV: O^T_unnorm[65, q] = [V | 1].T @ P^T over k tiles; row 64 = l.
  7. l-row -> SBUF (ACT copy) -> reciprocal_approx_fast (DVE) -> gpsimd
     partition_broadcast -> DVE multiply => normalized O^T in Ocat (fp16).
  8. proj: Y[q, 768] = O^T.T @ projT (fp16) -> fp32 partial out.
"""

import os
import sys

import numpy as np

for _p in ("/opt/trn_rl_repo", "/root/.axon_site/_ro/trn_rl_repo"):
    if os.path.isdir(_p) and _p not in sys.path:
        sys.path.insert(0, _p)

import ml_dtypes  # noqa: E402

import concourse.mybir as mybir  # noqa: E402
from concourse import bacc  # noqa: E402
from concourse.bass_utils import run_bass_kernel_spmd  # noqa: E402
from concourse.masks import make_identity  # noqa: E402
from concourse.tile import TileContext  # noqa: E402

B, N, C, H = 4, 2048, 768, 12
D = C // H          # 64
HG = 2              # head groups (cores per batch)
HPC = H // HG       # heads per core = 6
CIN_T = C // 128    # 6 cin tiles
QT_TILES = 3        # head pairs
KT_TILES = N // 128  # 16
NCORES = 8
MASK_BIAS = -1048576.0  # -2^20, exact in bf16; scores are already x64

F32 = mybir.dt.float32
F32R = mybir.dt.float32r
F16 = mybir.dt.float16
BF16 = mybir.dt.bfloat16

_CACHE = {}


def _build_program():
    variant = os.environ.get("KVARIANT", "full")
    nc = bacc.Bacc(
        "TRN2",
        target_bir_lowering=False,
        debug=False,
        enable_asserts=False,
        num_devices=NCORES,
    )
    xT = nc.dram_tensor("xT", [C, N], F32R, kind="ExternalInput").ap()
    qkvT = nc.dram_tensor("qkvT", [C, 3 * HPC * D], F32R, kind="ExternalInput").ap()
    maskb = nc.dram_tensor("maskb", [N, N], BF16, kind="ExternalInput").ap()
    projT = nc.dram_tensor("projT", [HPC * D, C], F16, kind="ExternalInput").ap()
    out = nc.dram_tensor("out", [N, C], F32, kind="ExternalOutput").ap()

    AL = mybir.AluOpType

    with TileContext(nc) as tc:
        with tc.tile_pool(name="pers", bufs=1) as pers:
            # ---- persistent tiles ----
            QTs = [
                pers.tile([128, N], F32R, tag=f"qt{t}", name=f"qt{t}")
                for t in range(QT_TILES)
            ]
            KTs = [
                pers.tile([128, N], F32R, tag=f"kt{t}", name=f"kt{t}")
                for t in range(QT_TILES)
            ]
            # V augmented with a ones column: [128, (h,kt), 65]
            Vaug = pers.tile([128, HPC * KT_TILES, D + 1], F16, tag="vaug")
            Ocat = [
                pers.tile([128, N], F16, tag=f"oc{t}", name=f"oc{t}")
                for t in range(QT_TILES)
            ]
            PW = [
                pers.tile([128, C], F16, tag=f"pw{t}", name=f"pw{t}")
                for t in range(QT_TILES)
            ]
            ident = pers.tile([128, 128], BF16, tag="ident")

            make_identity(nc, ident[:, :])
            nc.gpsimd.memset(Vaug[:, :, D : D + 1], 1.0)
            for t in range(QT_TILES):
                nc.sync.dma_start(PW[t][:, :], projT[t * 128 : (t + 1) * 128, :])

            # ================= Phase 1: QKV projection =================
            with (
                tc.tile_pool(name="ph1", bufs=1) as p1,
                tc.tile_pool(name="ph1p", bufs=4, space="PSUM") as p1p,
            ):
                xts = [
                    p1.tile([128, N], F32R, tag=f"x{ci}", name=f"x{ci}")
                    for ci in range(CIN_T)
                ]
                wts = [
                    p1.tile([128, 3 * HPC * D], F32R, tag=f"w{ci}", name=f"w{ci}")
                    for ci in range(CIN_T)
                ]
                for ci in range(CIN_T):
                    nc.sync.dma_start(xts[ci][:, :], xT[ci * 128 : (ci + 1) * 128, :])
                    nc.scalar.dma_start(
                        wts[ci][:, :], qkvT[ci * 128 : (ci + 1) * 128, :]
                    )

                # Q^T and K^T production: out[d_tile 128, q 512]
                for which, dst in ((0, QTs), (1, KTs)):
                    off = which * HPC * D  # 0 or 384 within qkvT cols
                    for t in range(QT_TILES):
                        for qc in range(4):
                            ps = p1p.tile([128, 512], F32, tag="p1ps", name="ps")
                            for ci in range(CIN_T):
                                nc.tensor.matmul(
                                    ps[:, :],
                                    wts[ci][:, off + t * 128 : off + (t + 1) * 128],
                                    xts[ci][:, qc * 512 : (qc + 1) * 512],
                                    start=(ci == 0),
                                    stop=(ci == CIN_T - 1),
                                )
                            nc.vector.tensor_copy(
                                dst[t][:, qc * 512 : (qc + 1) * 512], ps[:, :]
                            )

                # V production: out[k_tile 128, 384] -> Vaug f16
                voff = 2 * HPC * D  # 768
                for kt in range(KT_TILES):
                    ps = p1p.tile([128, HPC * D], F32, tag="p1ps", name="ps")
                    for ci in range(CIN_T):
                        nc.tensor.matmul(
                            ps[:, :],
                            xts[ci][:, kt * 128 : (kt + 1) * 128],
                            wts[ci][:, voff : voff + HPC * D],
                            start=(ci == 0),
                            stop=(ci == CIN_T - 1),
                        )
                    # psum [128, (h 6, d 64)] -> Vaug[:, h*16+kt, 0:64]
                    nc.vector.tensor_copy(
                        Vaug[:, kt :: KT_TILES, 0:D],
                        ps[:, :].rearrange("p (h d) -> p h d", h=HPC),
                    )

            # ================= Phase 2: attention =================
            with (
                tc.tile_pool(name="mk", bufs=2) as pmk,
                tc.tile_pool(name="work", bufs=2) as pw,
                tc.tile_pool(name="psS", bufs=3, space="PSUM") as psS,
                tc.tile_pool(name="psO", bufs=2, space="PSUM") as psO,
            ):
                for qc in range(4):
                    mks = []
                    for j in range(4):
                        mk = pmk.tile([128, N], BF16, tag=f"mk{j}", name=f"mk{j}")
                        row0 = qc * 512 + j * 128
                        nc.scalar.dma_start(mk[:, :], maskb[row0 : row0 + 128, :])
                        mks.append(mk)

                    for hp in range(QT_TILES):  # head pairs
                        PTs = [
                            pw.tile(
                                [128, KT_TILES, 512], F16, tag="ptrans",
                                name=f"PT{a}", bufs=3,
                            )
                            for a in range(2)
                        ]
                        for j in range(4):
                            qt = qc * 4 + j
                            mstats = [
                                pw.tile([128, 2], F32, tag=f"mstat{a}",
                                        name=f"mstat{a}", bufs=6)
                                for a in range(2)
                            ]
                            pns = [
                                pw.tile([128, N], F16, tag=f"pn{a}",
                                        name=f"pn{a}", bufs=3)
                                for a in range(2)
                            ]
                            negms = [
                                pw.tile([128, 1], F32, tag=f"negm{a}",
                                        name=f"negm{a}", bufs=6)
                                for a in range(2)
                            ]
                            for half in range(2):
                                sps = [
                                    psS.tile([128, 1024], F32, tag="sp",
                                             name=f"sp{a}")
                                    for a in range(2)
                                ]
                                # packed K=64 score matmuls: head a in row
                                # group a (partitions 64a..64a+63)
                                for c in range(2):
                                    kc = half * 1024 + c * 512
                                    for a in range(2):
                                        nc.tensor.matmul(
                                            sps[a][:, c * 512 : (c + 1) * 512],
                                            QTs[hp][
                                                a * D : (a + 1) * D,
                                                qt * 128 : (qt + 1) * 128,
                                            ],
                                            KTs[hp][
                                                a * D : (a + 1) * D,
                                                kc : kc + 512,
                                            ],
                                            start=True,
                                            stop=False,
                                            tile_position=(a * D, 0),
                                        )
                                for c in range(2):
                                    kc = half * 1024 + c * 512
                                    for a in range(2):
                                        nc.tensor.matmul(
                                            sps[a][:, c * 512 : (c + 1) * 512],
                                            ident[:, :],
                                            mks[j][:, kc : kc + 512],
                                            start=False,
                                            stop=True,
                                        )
                                for a in range(2):
                                    nc.vector.tensor_reduce(
                                        mstats[a][:, half : half + 1],
                                        sps[a][:, :],
                                        axis=mybir.AxisListType.X,
                                        op=AL.max,
                                        negate=True,
                                    )
                                    if half == 0:
                                        # exp(S - m0) immediately; renorm later
                                        nc.scalar.activation(
                                            pns[a][:, 0:1024],
                                            sps[a][:, :],
                                            mybir.ActivationFunctionType.Exp,
                                            bias=mstats[a][:, 0:1],
                                            scale=1.0,
                                        )
                                    else:
                                        # negm = -M, then exp(S - M) directly
                                        nc.vector.tensor_reduce(
                                            negms[a][:, :],
                                            mstats[a][:, 0:2],
                                            axis=mybir.AxisListType.X,
                                            op=AL.min,
                                        )
                                        nc.scalar.activation(
                                            pns[a][:, 1024:2048],
                                            sps[a][:, :],
                                            mybir.ActivationFunctionType.Exp,
                                            bias=negms[a][:, 0:1],
                                            scale=1.0,
                                        )
                            for a in range(2):
                                # fs0 = exp(m0 - M); renormalize half0 only
                                fs = pw.tile([128, 1], F32, tag=f"fs{a}",
                                             name=f"fs{a}", bufs=6)
                                nc.scalar.activation(
                                    fs[:, :],
                                    mstats[a][:, 0:1],
                                    mybir.ActivationFunctionType.Exp,
                                    bias=negms[a][:, 0:1],
                                    scale=-1.0,
                                )
                                nc.vector.tensor_scalar(
                                    pns[a][:, 0:1024],
                                    pns[a][:, 0:1024],
                                    fs[:, 0:1],
                                    None,
                                    op0=AL.mult,
                                )
                                nc.sync.dma_start_transpose(
                                    PTs[a][:, :, j * 128 : (j + 1) * 128],
                                    pns[a][:, :],
                                )

                        for a in range(2):
                            h = 2 * hp + a
                            # PV: O^T_unnorm [65, 512q]; row 64 = l
                            ot = psO.tile([D + 1, 512], F32, tag="ot", name="ot")
                            for kt in range(KT_TILES):
                                nc.tensor.matmul(
                                    ot[:, :],
                                    Vaug[:, h * KT_TILES + kt, :],
                                    PTs[a][:, kt, :],
                                    start=(kt == 0),
                                    stop=(kt == KT_TILES - 1),
                                )
                            lrow = pw.tile([1, 512], F32, tag="lrow",
                                           name="lrow", bufs=2)
                            nc.scalar.copy(lrow[:, :], ot[D : D + 1, :])
                            rl = pw.tile([1, 512], F32, tag="rl", name="rl",
                                         bufs=2)
                            if variant == "slowrecip":
                                nc.vector.reciprocal(rl[:, :], lrow[:, :])
                            else:
                                nc.vector.reciprocal_approx_fast(
                                    rl[:, :], lrow[:, :]
                                )
                            rb = pw.tile([D, 512], F32, tag="rb", name="rb",
                                         bufs=2)
                            nc.gpsimd.partition_broadcast(rb[:, :], rl[:, :])
                            nc.vector.tensor_tensor(
                                Ocat[hp][a * D : (a + 1) * D,
                                         qc * 512 : (qc + 1) * 512],
                                ot[0:D, :],
                                rb[:, :],
                                op=AL.mult,
                            )

                    # proj for this q-chunk's 4 q-tiles
                    for j in range(4):
                        qt = qc * 4 + j
                        y0 = psO.tile([128, 512], F32, tag="ot", name="y0")
                        y1 = psO.tile([128, 256], F32, tag="ot", name="y1")
                        for ct in range(QT_TILES):
                            lt = Ocat[ct][:, qt * 128 : (qt + 1) * 128]
                            nc.tensor.matmul(
                                y0[:, :],
                                lt,
                                PW[ct][:, 0:512],
                                start=(ct == 0),
                                stop=(ct == QT_TILES - 1),
                            )
                            nc.tensor.matmul(
                                y1[:, :],
                                lt,
                                PW[ct][:, 512:768],
                                start=(ct == 0),
                                stop=(ct == QT_TILES - 1),
                            )
                        ysb = pw.tile([128, C], F32, tag="ysb", name="ysb")
                        nc.scalar.copy(ysb[:, 0:512], y0[:, :])
                        nc.scalar.copy(ysb[:, 512:768], y1[:, :])
                        nc.sync.dma_start(
                            out[qt * 128 : (qt + 1) * 128, :], ysb[:, :]
                        )
    nc.compile()
    return nc


def kernel(x, local_attn_mask, qkv_w, proj_w, proj_b):
    x = np.asarray(x, dtype=np.float32)
    mask = np.asarray(local_attn_mask)
    qkv_w = np.asarray(qkv_w, dtype=np.float32)
    proj_w = np.asarray(proj_w, dtype=np.float32)
    proj_b = np.asarray(proj_b, dtype=np.float32)

    maskb = (MASK_BIAS * mask.astype(np.float32)).astype(ml_dtypes.bfloat16)
    in_maps = []
    for c in range(NCORES):
        b, hg = c // HG, c % HG
        rq = slice(hg * HPC * D, (hg + 1) * HPC * D)
        rk = slice(C + hg * HPC * D, C + (hg + 1) * HPC * D)
        rv = slice(2 * C + hg * HPC * D, 2 * C + (hg + 1) * HPC * D)
        # softmax scale D folded into the Q weights
        wsel = np.concatenate(
            [qkv_w[rq] * float(D), qkv_w[rk], qkv_w[rv]], axis=0
        )  # [1152, 768]
        in_maps.append(
            {
                "xT": np.ascontiguousarray(x[b].T),
                "qkvT": np.ascontiguousarray(wsel.T),
                "maskb": maskb,
                "projT": np.ascontiguousarray(
                    proj_w[:, hg * HPC * D : (hg + 1) * HPC * D].T
                ).astype(np.float16),
            }
        )

    if "nc" not in _CACHE:
        _CACHE["nc"] = _build_program()
    res = run_bass_kernel_spmd(_CACHE["nc"], in_maps, core_ids=list(range(NCORES)))
    _CACHE["res"] = res
    outs = res.results
    y = np.empty((B, N, C), dtype=np.float32)
    for b in range(B):
        y[b] = outs[2 * b]["out"] + outs[2 * b + 1]["out"] + proj_b[None, :]
    return y


# revision 17
# speedup vs baseline: 1.1310x; 1.1310x over previous
"""
Trainium2 Bass kernel for nn_Attention_29265907155069.

Reference computation (B=4, N=2048, C=768, H=12, D=64):
    qkv = x @ qkv_w.T -> split to q,k,v per head
    attn = softmax(q @ k.T * D + mask * -1e6)
    out  = (attn @ v) re-concat -> @ proj_w.T + proj_b
Sharding: 8 cores = (batch b in 0..3) x (head-group hg in 0..1, 6 heads each).
Host sums the two head-group partials (row-sharded proj).

Per-core pipeline (v3):
  1. QKV: Q^T,K^T [d, n] and V [k, d] via PE matmuls (float32r); D=64 scale
     folded into Q weights on host. PSUM->SBUF copies on DVE (idle in ph 1).
  2. Scores S per (head a, q-tile, k-half) into PSUM [128, 1024] (two heads
     packed via K=64 row-tiling); identity matmul accumulates -2^20*mask.
  3. DVE reduce_max(negate) per half -> -m_half.  negm = -M = min of both,
     available right after half1's reduce at no extra latency.
  4. ACT exp: half0 biased by -m0 (fires immediately, frees its PSUM banks
     fast); half1 biased by negm.  Only half0 then needs the fs = exp(m0-M)
     renormalize multiply (DVE tensor_scalar, fp16 4x) - half the v1 cost.
  5. DMA xbar transpose per (head, q-tile): pns [128, 2048] -> PT.
  6. PV: O^T_unnorm[65, q] = [V | 1].T @ P^T over k tiles; row 64 = l.
  7. l-row -> SBUF (ACT copy) -> reciprocal_approx_fast (DVE) -> gpsimd
     partition_broadcast -> DVE multiply => normalized O^T in Ocat (fp16).
  8. proj: Y[q, 768] = O^T.T @ projT (fp16) -> fp32 partial out.
"""

import os
import sys

import numpy as np

for _p in ("/opt/trn_rl_repo", "/root/.axon_site/_ro/trn_rl_repo"):
    if os.path.isdir(_p) and _p not in sys.path:
        sys.path.insert(0, _p)

import ml_dtypes  # noqa: E402

import concourse.mybir as mybir  # noqa: E402
from concourse import bacc  # noqa: E402
from concourse.bass_utils import run_bass_kernel_spmd  # noqa: E402
from concourse.masks import make_identity  # noqa: E402
from concourse.tile import TileContext  # noqa: E402

B, N, C, H = 4, 2048, 768, 12
D = C // H          # 64
HG = 2              # head groups (cores per batch)
HPC = H // HG       # heads per core = 6
CIN_T = C // 128    # 6 cin tiles
QT_TILES = 3        # head pairs
KT_TILES = N // 128  # 16
NCORES = 8
MASK_BIAS = -1048576.0  # -2^20, exact in bf16; scores are already x64

F32 = mybir.dt.float32
F32R = mybir.dt.float32r
F16 = mybir.dt.float16
BF16 = mybir.dt.bfloat16

_CACHE = {}


def _build_program():
    variant = os.environ.get("KVARIANT", "full")
    nc = bacc.Bacc(
        "TRN2",
        target_bir_lowering=False,
        debug=False,
        enable_asserts=False,
        num_devices=NCORES,
    )
    xT = nc.dram_tensor("xT", [C, N], F32R, kind="ExternalInput").ap()
    qkvT = nc.dram_tensor("qkvT", [C, 3 * HPC * D], F32R, kind="ExternalInput").ap()
    maskb = nc.dram_tensor("maskb", [N, N], BF16, kind="ExternalInput").ap()
    projT = nc.dram_tensor("projT", [HPC * D, C], F16, kind="ExternalInput").ap()
    out = nc.dram_tensor("out", [N, C], F32, kind="ExternalOutput").ap()

    AL = mybir.AluOpType

    with TileContext(nc) as tc:
        with tc.tile_pool(name="pers", bufs=1) as pers:
            # ---- persistent tiles ----
            QTs = [
                pers.tile([128, N], F32R, tag=f"qt{t}", name=f"qt{t}")
                for t in range(QT_TILES)
            ]
            KTs = [
                pers.tile([128, N], F32R, tag=f"kt{t}", name=f"kt{t}")
                for t in range(QT_TILES)
            ]
            # V augmented with a ones column: [128, (h,kt), 65]
            Vaug = pers.tile([128, HPC * KT_TILES, D + 1], F16, tag="vaug")
            Ocat = [
                pers.tile([128, N], F16, tag=f"oc{t}", name=f"oc{t}")
                for t in range(QT_TILES)
            ]
            PW = [
                pers.tile([128, C], F16, tag=f"pw{t}", name=f"pw{t}")
                for t in range(QT_TILES)
            ]
            ident = pers.tile([128, 128], BF16, tag="ident")

            make_identity(nc, ident[:, :])
            nc.gpsimd.memset(Vaug[:, :, D : D + 1], 1.0)
            for t in range(QT_TILES):
                nc.sync.dma_start(PW[t][:, :], projT[t * 128 : (t + 1) * 128, :])

            # ================= Phase 1: QKV projection =================
            with (
                tc.tile_pool(name="ph1", bufs=1) as p1,
                tc.tile_pool(name="ph1p", bufs=4, space="PSUM") as p1p,
            ):
                xts = [
                    p1.tile([128, N], F32R, tag=f"x{ci}", name=f"x{ci}")
                    for ci in range(CIN_T)
                ]
                wts = [
                    p1.tile([128, 3 * HPC * D], F32R, tag=f"w{ci}", name=f"w{ci}")
                    for ci in range(CIN_T)
                ]
                for ci in range(CIN_T):
                    nc.sync.dma_start(xts[ci][:, :], xT[ci * 128 : (ci + 1) * 128, :])
                    nc.scalar.dma_start(
                        wts[ci][:, :], qkvT[ci * 128 : (ci + 1) * 128, :]
                    )

                # Q^T and K^T production: out[d_tile 128, q 512].
                # Per head-pair t emit Q then K so phase 2's head-pair 0
                # unblocks after 1/3 of the Q/K work instead of after all
                # Q tiles.
                for t in range(QT_TILES):
                    for which, dst in ((0, QTs), (1, KTs)):
                        off = which * HPC * D  # 0 or 384 within qkvT cols
                        for qc in range(4):
                            ps = p1p.tile([128, 512], F32, tag="p1ps", name="ps")
                            for ci in range(CIN_T):
                                nc.tensor.matmul(
                                    ps[:, :],
                                    wts[ci][:, off + t * 128 : off + (t + 1) * 128],
                                    xts[ci][:, qc * 512 : (qc + 1) * 512],
                                    start=(ci == 0),
                                    stop=(ci == CIN_T - 1),
                                )
                            nc.vector.tensor_copy(
                                dst[t][:, qc * 512 : (qc + 1) * 512], ps[:, :]
                            )

                # V production: out[k_tile 128, 384] -> Vaug f16
                voff = 2 * HPC * D  # 768
                for kt in range(KT_TILES):
                    ps = p1p.tile([128, HPC * D], F32, tag="p1ps", name="ps")
                    for ci in range(CIN_T):
                        nc.tensor.matmul(
                            ps[:, :],
                            xts[ci][:, kt * 128 : (kt + 1) * 128],
                            wts[ci][:, voff : voff + HPC * D],
                            start=(ci == 0),
                            stop=(ci == CIN_T - 1),
                        )
                    # psum [128, (h 6, d 64)] -> Vaug[:, h*16+kt, 0:64]
                    nc.vector.tensor_copy(
                        Vaug[:, kt :: KT_TILES, 0:D],
                        ps[:, :].rearrange("p (h d) -> p h d", h=HPC),
                    )

            # ================= Phase 2: attention =================
            with (
                tc.tile_pool(name="mk", bufs=2) as pmk,
                tc.tile_pool(name="work", bufs=2) as pw,
                tc.tile_pool(name="psS", bufs=3, space="PSUM") as psS,
                tc.tile_pool(name="psO", bufs=2, space="PSUM") as psO,
            ):
                for qc in range(4):
                    mks = []
                    for j in range(4):
                        mk = pmk.tile([128, N], BF16, tag=f"mk{j}", name=f"mk{j}")
                        row0 = qc * 512 + j * 128
                        nc.scalar.dma_start(mk[:, :], maskb[row0 : row0 + 128, :])
                        mks.append(mk)

                    for hp in range(QT_TILES):  # head pairs
                        PTs = [
                            pw.tile(
                                [128, KT_TILES, 512], F16, tag="ptrans",
                                name=f"PT{a}", bufs=3,
                            )
                            for a in range(2)
                        ]
                        for j in range(4):
                            qt = qc * 4 + j
                            mstats = [
                                pw.tile([128, 2], F32, tag=f"mstat{a}",
                                        name=f"mstat{a}", bufs=6)
                                for a in range(2)
                            ]
                            pns = [
                                pw.tile([128, N], F16, tag=f"pn{a}",
                                        name=f"pn{a}", bufs=3)
                                for a in range(2)
                            ]
                            negms = [
                                pw.tile([128, 1], F32, tag=f"negm{a}",
                                        name=f"negm{a}", bufs=6)
                                for a in range(2)
                            ]
                            for half in range(2):
                                sps = [
                                    psS.tile([128, 1024], F32, tag="sp",
                                             name=f"sp{a}")
                                    for a in range(2)
                                ]
                                # packed K=64 score matmuls: head a in row
                                # group a (partitions 64a..64a+63)
                                for c in range(2):
                                    kc = half * 1024 + c * 512
                                    for a in range(2):
                                        nc.tensor.matmul(
                                            sps[a][:, c * 512 : (c + 1) * 512],
                                            QTs[hp][
                                                a * D : (a + 1) * D,
                                                qt * 128 : (qt + 1) * 128,
                                            ],
                                            KTs[hp][
                                                a * D : (a + 1) * D,
                                                kc : kc + 512,
                                            ],
                                            start=True,
                                            stop=False,
                                            tile_position=(a * D, 0),
                                        )
                                for c in range(2):
                                    kc = half * 1024 + c * 512
                                    for a in range(2):
                                        nc.tensor.matmul(
                                            sps[a][:, c * 512 : (c + 1) * 512],
                                            ident[:, :],
                                            mks[j][:, kc : kc + 512],
                                            start=False,
                                            stop=True,
                                        )
                                for a in range(2):
                                    nc.vector.tensor_reduce(
                                        mstats[a][:, half : half + 1],
                                        sps[a][:, :],
                                        axis=mybir.AxisListType.X,
                                        op=AL.max,
                                        negate=True,
                                    )
                                    if half == 0:
                                        # exp(S - m0) immediately; renorm later
                                        nc.scalar.activation(
                                            pns[a][:, 0:1024],
                                            sps[a][:, :],
                                            mybir.ActivationFunctionType.Exp,
                                            bias=mstats[a][:, 0:1],
                                            scale=1.0,
                                        )
                                    else:
                                        # negm = -M, then exp(S - M) directly
                                        nc.vector.tensor_reduce(
                                            negms[a][:, :],
                                            mstats[a][:, 0:2],
                                            axis=mybir.AxisListType.X,
                                            op=AL.min,
                                        )
                                        nc.scalar.activation(
                                            pns[a][:, 1024:2048],
                                            sps[a][:, :],
                                            mybir.ActivationFunctionType.Exp,
                                            bias=negms[a][:, 0:1],
                                            scale=1.0,
                                        )
                            for a in range(2):
                                # fs0 = exp(m0 - M); renormalize half0 only
                                fs = pw.tile([128, 1], F32, tag=f"fs{a}",
                                             name=f"fs{a}", bufs=6)
                                nc.scalar.activation(
                                    fs[:, :],
                                    mstats[a][:, 0:1],
                                    mybir.ActivationFunctionType.Exp,
                                    bias=negms[a][:, 0:1],
                                    scale=-1.0,
                                )
                                nc.vector.tensor_scalar(
                                    pns[a][:, 0:1024],
                                    pns[a][:, 0:1024],
                                    fs[:, 0:1],
                                    None,
                                    op0=AL.mult,
                                )
                                nc.sync.dma_start_transpose(
                                    PTs[a][:, :, j * 128 : (j + 1) * 128],
                                    pns[a][:, :],
                                )

                        for a in range(2):
                            h = 2 * hp + a
                            # PV: O^T_unnorm [65, 512q]; row 64 = l
                            ot = psO.tile([D + 1, 512], F32, tag="ot", name="ot")
                            for kt in range(KT_TILES):
                                nc.tensor.matmul(
                                    ot[:, :],
                                    Vaug[:, h * KT_TILES + kt, :],
                                    PTs[a][:, kt, :],
                                    start=(kt == 0),
                                    stop=(kt == KT_TILES - 1),
                                )
                            lrow = pw.tile([1, 512], F32, tag="lrow",
                                           name="lrow", bufs=2)
                            nc.scalar.copy(lrow[:, :], ot[D : D + 1, :])
                            rl = pw.tile([1, 512], F32, tag="rl", name="rl",
                                         bufs=2)
                            if variant == "slowrecip":
                                nc.vector.reciprocal(rl[:, :], lrow[:, :])
                            else:
                                nc.vector.reciprocal_approx_fast(
                                    rl[:, :], lrow[:, :]
                                )
                            rb = pw.tile([D, 512], F32, tag="rb", name="rb",
                                         bufs=2)
                            nc.gpsimd.partition_broadcast(rb[:, :], rl[:, :])
                            nc.vector.tensor_tensor(
                                Ocat[hp][a * D : (a + 1) * D,
                                         qc * 512 : (qc + 1) * 512],
                                ot[0:D, :],
                                rb[:, :],
                                op=AL.mult,
                            )

                    # proj for this q-chunk's 4 q-tiles
                    for j in range(4):
                        qt = qc * 4 + j
                        y0 = psO.tile([128, 512], F32, tag="ot", name="y0")
                        y1 = psO.tile([128, 256], F32, tag="ot", name="y1")
                        for ct in range(QT_TILES):
                            lt = Ocat[ct][:, qt * 128 : (qt + 1) * 128]
                            nc.tensor.matmul(
                                y0[:, :],
                                lt,
                                PW[ct][:, 0:512],
                                start=(ct == 0),
                                stop=(ct == QT_TILES - 1),
                            )
                            nc.tensor.matmul(
                                y1[:, :],
                                lt,
                                PW[ct][:, 512:768],
                                start=(ct == 0),
                                stop=(ct == QT_TILES - 1),
                            )
                        ysb = pw.tile([128, C], F32, tag="ysb", name="ysb")
                        nc.scalar.copy(ysb[:, 0:512], y0[:, :])
                        nc.scalar.copy(ysb[:, 512:768], y1[:, :])
                        nc.sync.dma_start(
                            out[qt * 128 : (qt + 1) * 128, :], ysb[:, :]
                        )
    nc.compile()
    return nc


def kernel(x, local_attn_mask, qkv_w, proj_w, proj_b):
    x = np.asarray(x, dtype=np.float32)
    mask = np.asarray(local_attn_mask)
    qkv_w = np.asarray(qkv_w, dtype=np.float32)
    proj_w = np.asarray(proj_w, dtype=np.float32)
    proj_b = np.asarray(proj_b, dtype=np.float32)

    maskb = (MASK_BIAS * mask.astype(np.float32)).astype(ml_dtypes.bfloat16)
    in_maps = []
    for c in range(NCORES):
        b, hg = c // HG, c % HG
        rq = slice(hg * HPC * D, (hg + 1) * HPC * D)
        rk = slice(C + hg * HPC * D, C + (hg + 1) * HPC * D)
        rv = slice(2 * C + hg * HPC * D, 2 * C + (hg + 1) * HPC * D)
        # softmax scale D folded into the Q weights
        wsel = np.concatenate(
            [qkv_w[rq] * float(D), qkv_w[rk], qkv_w[rv]], axis=0
        )  # [1152, 768]
        in_maps.append(
            {
                "xT": np.ascontiguousarray(x[b].T),
                "qkvT": np.ascontiguousarray(wsel.T),
                "maskb": maskb,
                "projT": np.ascontiguousarray(
                    proj_w[:, hg * HPC * D : (hg + 1) * HPC * D].T
                ).astype(np.float16),
            }
        )

    if "nc" not in _CACHE:
        _CACHE["nc"] = _build_program()
    res = run_bass_kernel_spmd(_CACHE["nc"], in_maps, core_ids=list(range(NCORES)))
    _CACHE["res"] = res
    outs = res.results
    y = np.empty((B, N, C), dtype=np.float32)
    for b in range(B):
        y[b] = outs[2 * b]["out"] + outs[2 * b + 1]["out"] + proj_b[None, :]
    return y
